# revision 26
# baseline (speedup 1.0000x reference)
"""Trainium2 Bass kernel for CorrelatedCategoricalsLM (GRU LM).

Sharding: data-parallel over batch across 8 NeuronCores (4 rows each).
On-chip layout is "T-layout": feature dims live on SBUF partitions, the
4*T token axis (t-major: tok = 4*t + b) lives on the free axis, so the
element-wise GRU gate math runs with all 128 lanes busy.

Matmuls run as float32r (full-rate fp32 on the PE array).
"""

import sys

sys.path.insert(0, "/opt/trn_rl_repo")

import numpy as np

B, T, V, E, H, DZ = 32, 128, 32000, 512, 512, 256
NCORES = 8
BL = B // NCORES            # local batch rows per core
G3 = 3 * H                  # gate rows (r, z, n)
EC = E // 128               # embedding feature chunks
DZC = DZ // 128             # z feature chunks
KE = (E + DZ) // 128        # rnn-input feature chunks
KH = H // 128               # hidden feature chunks
MG = G3 // 128              # gate m-tiles


def _build_nc(T_=T, V_=V, vg=10, use_f32r=True, bf_rec=False, bf_out=False, skip_bn=False):
    """Build the single-core Bass/Tile program (SPMD: same program, 8 cores)."""
    import concourse.mybir as mybir
    import concourse.tile as tile
    from concourse import bacc

    dt = mybir.dt
    f32 = dt.float32
    bf16 = dt.bfloat16
    wh_dt = bf16 if bf_rec else f32
    wo_dt = bf16 if bf_out else f32
    AF = mybir.ActivationFunctionType

    TOK = BL * T_               # tokens per core
    TOKC = TOK // 128           # token tiles
    VT = V_ // 128              # vocab tiles
    NG = VT // vg               # vocab tile groups
    assert VT % vg == 0 and TOK % 128 == 0

    mm32 = dt.float32r if use_f32r else f32

    nc = bacc.Bacc("TRN2")

    # --- DRAM I/O (per core) ---
    xi = nc.dram_tensor("xi", [128, TOK // 16], dt.int16, kind="ExternalInput")
    zwi = nc.dram_tensor("zwi", [DZ, BL + H], mm32, kind="ExternalInput")  # [z.T | W_init.T]
    Wih = nc.dram_tensor("Wih", [E + DZ, G3], mm32, kind="ExternalInput")  # W_ih.T
    Whh = nc.dram_tensor("Whh", [H, G3], wh_dt, kind="ExternalInput")     # W_hh.T
    # cst cols: [identity(128) | b_init(KH) | b_ih+b_hh[r,z](MG) | b_hh[n](KH)]
    cst = nc.dram_tensor("cst", [128, 128 + KH + MG + KH], f32, kind="ExternalInput")
    emb = nc.dram_tensor("emb", [V_, E], f32, kind="ExternalInput")
    WoT = nc.dram_tensor("WoT", [H, V_], wo_dt, kind="ExternalInput")     # W_out.T
    logT = nc.dram_tensor("logT", [V_, TOK], wo_dt, kind="ExternalOutput")

    with tile.TileContext(nc) as tc:
        with tc.tile_pool(name="hs", bufs=1) as hsp:
            # hsT[:, k, BL*t : BL*(t+1)] = h_t.T chunk k (h_0 at t=0)
            hsT = hsp.tile([128, KH, BL * (T_ + 1)], f32)
            need_hsbf = bf_rec or bf_out
            hsB = hsp.tile([128, KH, BL * (T_ + 1)], bf16, name="hsB") if need_hsbf else hsT
            rec_h = hsB if bf_rec else hsT   # rhs source for the recurrence MMs
            out_h = hsB if bf_out else hsT   # rhs source for the projection MMs

            _wo_cm = tc.tile_pool(name="wo", bufs=2)
            _st_cm = tc.tile_pool(name="st", bufs=2)
            wop = _wo_cm.__enter__()
            stp = _st_cm.__enter__()
            with (
                tc.tile_pool(name="cst", bufs=1) as cstp,
                tc.tile_pool(name="gi", bufs=1) as gip,
                tc.tile_pool(name="whh", bufs=1) as whp,
            ):
                giT = gip.tile([128, MG, TOK], f32)
                bnb = cstp.tile([128, KH, BL], f32)   # b_hh(n) broadcast over b
                whh_s = whp.tile([128, KH, G3], wh_dt)
                nc.sync.dma_start(whh_s[:, :, :], Whh.ap().rearrange("(k p) g -> p k g", p=128))

                # ---------- phase 0: gather, transposes, h0, gi ----------
                with (
                    tc.tile_pool(name="pre", bufs=1) as prep,
                    tc.tile_pool(name="psP", bufs=1, space="PSUM") as psP,
                ):
                    cst_t = prep.tile([128, 128 + KH + MG + KH], f32)
                    nc.sync.dma_start(cst_t[:, :], cst.ap()[:, :])
                    ident = cst_t[:, 0:128]
                    bi_s = cst_t[:, 128:128 + KH]
                    bg_s = cst_t[:, 128 + KH:128 + KH + MG]
                    bn_s = cst_t[:, 128 + KH + MG:128 + KH + MG + KH]
                    for j in range(BL):
                        nc.vector.tensor_copy(bnb[:, :, j], bn_s[:, :])
                    idx_t = prep.tile([128, TOK // 16], dt.int16)
                    nc.sync.dma_start(idx_t[:, :], xi.ap()[:, :])
                    zwi_t = prep.tile([128, DZC, BL + H], mm32)
                    nc.sync.dma_start(zwi_t[:, :, :], zwi.ap().rearrange("(k p) c -> p k c", p=128))
                    wih_s = prep.tile([128, KE, G3], mm32)
                    nc.sync.dma_start(wih_s[:, :, :], Wih.ap().rearrange("(k p) g -> p k g", p=128))

                    # h0 = tanh(W_init @ z.T + b_init), built directly in T-layout
                    h0p = psP.tile([128, KH * BL], f32, bufs=1)
                    for m in range(KH):
                        for k in range(DZC):
                            nc.tensor.matmul(
                                h0p[:, m * BL:(m + 1) * BL],
                                lhsT=zwi_t[:, k, BL + 128 * m:BL + 128 * (m + 1)],
                                rhs=zwi_t[:, k, 0:BL],
                                start=(k == 0),
                                stop=(k == DZC - 1),
                            )
                    for m in range(KH):
                        nc.scalar.activation(
                            hsT[:, m, 0:BL], h0p[:, m * BL:(m + 1) * BL],
                            AF.Tanh, bias=bi_s[:, m:m + 1],
                        )
                    if need_hsbf:
                        nc.vector.tensor_copy(hsB[:, :, 0:BL], hsT[:, :, 0:BL])

                    # embedding gather: xe[p, c, :] = emb[idx[c*128+p], :]
                    xe = prep.tile([128, TOKC, E], f32)
                    nc.gpsimd.dma_gather(
                        out_ap=xe[:, :, :],
                        in_ap=emb.ap()[:, :],
                        idxs_ap=idx_t[:, :],
                        num_idxs=TOK,
                        num_idxs_reg=TOK,
                        elem_size=E,
                    )

                    # rnn_inT: chunks 0..EC-1 = x_embed.T, chunks EC.. = z.T repeated
                    rT = prep.tile([128, KE, TOK], mm32)
                    for hh in range(EC):
                        for c in range(TOKC):
                            tp = psP.tile([128, 128], f32, name="tp", bufs=4)
                            nc.tensor.transpose(
                                tp[:, :], xe[:, c, 128 * hh:128 * (hh + 1)], ident
                            )
                            nc.vector.tensor_copy(rT[:, hh, 128 * c:128 * (c + 1)], tp[:, :])
                    nc.vector.tensor_copy(rT[:, EC:KE, 0:BL], zwi_t[:, :, 0:BL])
                    w = BL
                    while w < TOK:
                        nc.vector.tensor_copy(rT[:, EC:KE, w:2 * w], rT[:, EC:KE, 0:w])
                        w *= 2

                    # giT = W_ih @ rnn_in.T + (b_ih + b_hh[r,z])
                    for m in range(MG):
                        pg = psP.tile([128, TOK], f32, name="pg", bufs=2)
                        for k in range(KE):
                            nc.tensor.matmul(
                                pg[:, :],
                                lhsT=wih_s[:, k, 128 * m:128 * (m + 1)],
                                rhs=rT[:, k, :],
                                start=(k == 0),
                                stop=(k == KE - 1),
                            )
                        nc.vector.tensor_scalar_add(giT[:, m, :], pg[:, :], bg_s[:, m:m + 1])

                # ---------- phase 1: GRU recurrence ----------
                with (
                    tc.tile_pool(name="psR", bufs=2, space="PSUM") as psR,
                    tc.tile_pool(name="recs", bufs=2) as recs,
                ):
                    for t in range(T_):
                        c0, c1 = BL * t, BL * (t + 1)
                        ph_rz = psR.tile([128, 8, BL], f32, name="ph_rz")
                        ph_n = psR.tile([128, KH, BL], f32, name="ph_n")
                        for m in range(MG):
                            out = ph_rz[:, m, :] if m < 8 else ph_n[:, m - 8, :]
                            for k in range(KH):
                                nc.tensor.matmul(
                                    out,
                                    lhsT=whh_s[:, k, 128 * m:128 * (m + 1)],
                                    rhs=rec_h[:, k, c0:c1],
                                    start=(k == 0),
                                    stop=(k == KH - 1),
                                )
                        a_rz = recs.tile([128, 8, BL], f32, name="a_rz")
                        nc.vector.tensor_add(a_rz[:, :, :], ph_rz[:, :, :], giT[:, 0:8, c0:c1])
                        rz = recs.tile([128, 8, BL], f32, name="rz")
                        nc.scalar.activation(rz[:, :, :], a_rz[:, :, :], AF.Sigmoid)
                        t1 = recs.tile([128, KH, BL], f32, name="t1")
                        if skip_bn:
                            nc.vector.tensor_mul(t1[:, :, :], rz[:, 0:4, :], ph_n[:, :, :])
                        else:
                            hn = recs.tile([128, KH, BL], f32, name="hn")
                            nc.vector.tensor_add(hn[:, :, :], ph_n[:, :, :], bnb[:, :, :])
                            nc.vector.tensor_mul(t1[:, :, :], rz[:, 0:4, :], hn[:, :, :])
                        t2 = recs.tile([128, KH, BL], f32, name="t2")
                        nc.vector.tensor_add(t2[:, :, :], t1[:, :, :], giT[:, 8:12, c0:c1])
                        nn = recs.tile([128, KH, BL], f32, name="nn")
                        nc.scalar.activation(nn[:, :, :], t2[:, :, :], AF.Tanh)
                        d = recs.tile([128, KH, BL], f32, name="d")
                        nc.vector.tensor_sub(d[:, :, :], hsT[:, :, c0:c1], nn[:, :, :])
                        e = recs.tile([128, KH, BL], f32, name="e")
                        nc.vector.tensor_mul(e[:, :, :], rz[:, 4:8, :], d[:, :, :])
                        if need_hsbf:
                            # bf16 state write feeds the next step's matmuls
                            nc.vector.tensor_add(hsB[:, :, c1:c1 + BL], nn[:, :, :], e[:, :, :])
                        # fp32 state (for the h_prev - n term) off the critical path
                        nc.vector.tensor_add(hsT[:, :, c1:c1 + BL], nn[:, :, :], e[:, :, :])

        # ---------- phase 2: vocab projection (logitsT = W_out @ hs.T) ----------
            WoT_r = WoT.ap().rearrange("(k p) (g j) -> g p k j", p=128, j=vg * 128)
            logT_r = logT.ap().rearrange("(g vl p) t -> g p vl t", p=128, vl=vg)
            with tc.tile_pool(name="psV", bufs=4, space="PSUM") as psV:
                for g in range(NG):
                    wg = wop.tile([128, KH, vg * 128], wo_dt, name="wg")
                    nc.sync.dma_start(wg[:, :, :], WoT_r[g])
                    st = stp.tile([128, vg, TOK], wo_dt, name="st")
                    for vl in range(vg):
                        pv = psV.tile([128, TOK], f32, name="pv")
                        for k in range(KH):
                            nc.tensor.matmul(
                                pv[:, :],
                                lhsT=wg[:, k, 128 * vl:128 * (vl + 1)],
                                rhs=out_h[:, k, BL:BL * (T_ + 1)],
                                start=(k == 0),
                                stop=(k == KH - 1),
                            )
                        if vl % 2 == 0:
                            nc.vector.tensor_copy(st[:, vl, :], pv[:, :])
                        else:
                            nc.scalar.copy(st[:, vl, :], pv[:, :])
                    nc.sync.dma_start(logT_r[g], st[:, :, :])
                _st_cm.__exit__(None, None, None)
                _wo_cm.__exit__(None, None, None)

    nc.compile()
    return nc


def _prep_core_inputs(x, z, emb, W_init, b_init, W_ih, W_hh, b_ih, b_hh, W_out,
                      T_=T, V_=V, bf_rec=False, bf_out=False):
    """Host-side prep: shard over batch, transpose weights, wrap indices."""
    import ml_dtypes

    f32 = np.float32
    bf = ml_dtypes.bfloat16
    WiT = np.ascontiguousarray(W_init.T, dtype=f32)
    WihT = np.ascontiguousarray(W_ih.T, dtype=f32)
    WhhT = np.ascontiguousarray(W_hh.T).astype(bf if bf_rec else f32)
    WoT = np.ascontiguousarray(W_out.T).astype(bf if bf_out else f32)
    embf = np.ascontiguousarray(emb, dtype=f32)
    bi_c = np.ascontiguousarray(b_init.reshape(KH, 128).T, dtype=f32)
    bg_c = np.ascontiguousarray(b_ih.reshape(MG, 128).T, dtype=f32).copy()
    bhh_c = np.ascontiguousarray(b_hh.reshape(MG, 128).T, dtype=f32)
    bg_c[:, 0:8] += bhh_c[:, 0:8]
    bn_c = np.ascontiguousarray(bhh_c[:, 8:12], dtype=f32)
    cst_c = np.ascontiguousarray(
        np.concatenate([np.eye(128, dtype=f32), bi_c, bg_c, bn_c], axis=1))

    in_maps = []
    ncores = x.shape[0] // BL
    for c in range(ncores):
        xl = x[c * BL:(c + 1) * BL]          # [BL, T]
        zl = z[c * BL:(c + 1) * BL]          # [BL, DZ]
        xs = np.ascontiguousarray(xl.T).reshape(-1)      # t-major: tok = BL*t + b
        xi16 = np.ascontiguousarray(np.tile(xs.reshape(-1, 16).T.astype(np.int16), (8, 1)))
        in_maps.append({
            "xi": xi16,
            "zwi": np.ascontiguousarray(
                np.concatenate([zl.T.astype(f32), WiT], axis=1)),
            "Wih": WihT, "Whh": WhhT, "cst": cst_c,
            "emb": embf, "WoT": WoT,
        })
    return in_maps


def _assemble_output(results, T_=T, V_=V):
    outs = []
    for res in results:
        lt = np.asarray(res["logT"]).astype(np.float32)   # [V, BL*T] tok-major cols
        lg = np.ascontiguousarray(lt.T).reshape(T_, BL, V_).transpose(1, 0, 2)
        outs.append(lg)
    return np.ascontiguousarray(np.concatenate(outs, axis=0), dtype=np.float32)


_NC_CACHE = {}


BF_REC = True    # bf16 W_hh + h in the recurrence matmuls (gates stay fp32)
BF_OUT = True    # bf16 W_out + hs in the vocab projection


def kernel(x, z, emb, W_init, b_init, W_ih, W_hh, b_ih, b_hh, W_out,
           _trace=False):
    from concourse.bass_utils import run_bass_kernel_spmd

    x = np.asarray(x)
    skip_bn = not np.asarray(b_hh)[2 * H:].any()
    key = ("full", BF_REC, BF_OUT, skip_bn)
    if key not in _NC_CACHE:
        _NC_CACHE[key] = _build_nc(bf_rec=BF_REC, bf_out=BF_OUT, skip_bn=skip_bn)
    nc = _NC_CACHE[key]
    in_maps = _prep_core_inputs(
        x, np.asarray(z), np.asarray(emb), np.asarray(W_init), np.asarray(b_init),
        np.asarray(W_ih), np.asarray(W_hh), np.asarray(b_ih), np.asarray(b_hh),
        np.asarray(W_out), bf_rec=BF_REC, bf_out=BF_OUT,
    )
    res = run_bass_kernel_spmd(
        nc, in_maps, core_ids=list(range(NCORES)), trace=_trace,
    )
    out = _assemble_output(res.results)
    if _trace:
        return out, res
    return out
